# revision 21
# baseline (speedup 1.0000x reference)
"""GIN (3-layer) message-passing kernel for Trainium2, 8 NeuronCores.

v2 — batched-gather rewrite of the graph-partition data-parallel design.

  - Graphs assigned to cores by id (750 graphs x 50 nodes per core); nodes
    renumbered into a chunk-interleaved shared h table (4 chunks per layer
    for overlapped AllGathers).  The GIN self term comes from an
    SBUF-resident feat-major copy (zkeep), BN is folded into the next
    layer's first matmul (scale + rank-1 degree correction), stats come
    free from activation accum_out and a 1KB AllReduce.
  - Aggregation (the v2 part): edges are sharded by destination core and
    grouped by (512-slot destination group, 32768-row source window).
    Each (cohort of 3 groups, window) produces one int16 dma_gather call
    (<=1024 indices) instead of per-128-edge indirect DMAs: the SWDGE
    offset walker only supports one offset column per indirect call
    (~1.3us/call measured), while dma_gather moves ~5-6ns/row.  Gathered
    k-tiles (128 edges) are reduced into per-group PSUM banks by one-hot
    matmuls ([128e,128f]^T x [128e,512slots], is_equal-built one-hots).
  - MLP runs in transposed space per 512-col group (fp32r), pooling
    on-the-fly from raw m2 (max commutes with the final monotone affine),
    transposes feed the next layer's node-major h table.
Host assembles the 8 per-core [750, 384] outputs into the full [6000, 384].
"""

import sys

sys.path.insert(0, "/opt/trn_rl_repo")

import math
from dataclasses import dataclass, field

import numpy as np

try:
    from ml_dtypes import bfloat16 as np_bf16
except ImportError:  # pragma: no cover
    import jax.numpy as _jnp

    np_bf16 = _jnp.bfloat16

N_GRAPHS = 6000
N_CORES = 8
IN_DIM = 77
DIM = 128
EPS = 1e-5
GRP = 512  # slots per PSUM aggregation group
COH = 4  # groups per cohort (agg PSUM banks in flight)
NW = 10  # equal-size int16 gather windows (<=32768 rows each)
CAP_TILES = 8  # max k-tiles (128 idxs each) per dma_gather call
N_CHUNKS = 1  # single chunk: Shared DRAM allows one writer inst


@dataclass
class HostData:
    gs: int
    gpc: int
    slots: int
    nb: int
    shp: int
    wsz: int
    npairs: int
    cohorts: list  # per cohort: {'groups': [..], 'calls': [(w, icol0, tn, t0, pairs)]}
    pairs_per_group: np.ndarray  # [NG]
    icols: int
    idx16: list  # per core [128, icols] int16
    relp: list  # per core [128, npairs] f32
    degt: list  # per core [128, GRP] bf16
    x_tbl: np.ndarray  # [TBL, 128] bf16
    cb: np.ndarray  # chunk boundaries in blocks
    xT: list  # per core [128, SHP] bf16

    @property
    def tbl(self):
        return N_CORES * self.shp

    @property
    def ng(self):
        return (self.shp + GRP - 1) // GRP


def prep_host(x: np.ndarray, edge_index: np.ndarray, batch: np.ndarray) -> HostData:
    C = N_CORES
    N = x.shape[0]
    batch = batch.astype(np.int64)
    sizes = np.bincount(batch, minlength=N_GRAPHS)
    assert sizes.min() >= 1
    starts = np.concatenate([[0], np.cumsum(sizes)[:-1]])
    GS = int(sizes.max())
    GPC = N_GRAPHS // C
    SLOTS = GPC * GS
    NB = (SLOTS + 127) // 128
    SHP = NB * 128
    TBL = C * SHP
    NG = (SHP + GRP - 1) // GRP
    WSZ = ((TBL + NW - 1) // NW + 15) // 16 * 16
    assert WSZ <= 32768 and (NW - 1) * WSZ < TBL

    g_of = batch
    pos = np.arange(N, dtype=np.int64) - starts[g_of]
    core_of = g_of // GPC
    slot_loc = (g_of - core_of * GPC) * GS + pos
    row_of = (core_of * SHP + slot_loc).astype(np.int64)

    src = edge_index[0].astype(np.int64)
    dst = edge_index[1].astype(np.int64)

    e_core = [core_of[dst]]
    e_dslot = [slot_loc[dst]]
    e_srow = [row_of[src]]

    # duplicate slots: graph g's pad slots [size_g, GS) copy n0 = starts[g]
    n0_edges = np.where(dst == starts[g_of[dst]])[0]
    n0_g = g_of[dst[n0_edges]]
    max_pad = GS - int(sizes.min())
    for j in range(max_pad):
        gsel_mask = sizes + j < GS
        em = gsel_mask[n0_g]
        gg = n0_g[em]
        pc = gg // GPC
        ps = (gg - pc * GPC) * GS + sizes[gg] + j
        e_core.append(pc)
        e_dslot.append(ps)
        e_srow.append(row_of[src[n0_edges[em]]])

    e_core = np.concatenate(e_core)
    e_dslot = np.concatenate(e_dslot)
    e_srow = np.concatenate(e_srow)

    NCOH = (NG + COH - 1) // COH
    per_core = []
    cnt2 = np.zeros((C, NCOH, NW), dtype=np.int64)
    for c in range(C):
        m = e_core == c
        ds, sr = e_dslot[m], e_srow[m]
        g_e = ds // GRP
        coh_e = g_e // COH
        w_e = sr // WSZ
        order = np.lexsort((ds, w_e, coh_e))
        ds, sr, g_e, coh_e, w_e = (
            ds[order], sr[order], g_e[order], coh_e[order], w_e[order]
        )
        np.add.at(cnt2[c], (coh_e, w_e), 1)
        per_core.append((ds, sr, g_e))

    npad_cw = ((cnt2.max(axis=0) + 127) // 128) * 128  # [NCOH, NW]

    cell_pos = []
    for c in range(C):
        cc = cnt2[c].reshape(-1)
        cs = np.concatenate([[0], np.cumsum(cc)[:-1]]).reshape(NCOH, NW)
        cell_pos.append(cs)

    cohorts = []
    icol = 0
    pcol = 0
    pairs_per_group = np.zeros(NG, dtype=np.int64)
    group_seen = np.zeros(NG, dtype=bool)
    for ci in range(NCOH):
        groups = list(range(ci * COH, min((ci + 1) * COH, NG)))
        coh = {"groups": groups, "calls": []}
        for w in range(NW):
            npad = int(npad_cw[ci, w])
            if npad == 0:
                continue
            ntiles = npad // 128
            tile_groups = [dict() for _ in range(ntiles)]
            for c in range(C):
                ds, sr, g_e = per_core[c]
                s0 = cell_pos[c][ci, w]
                n_c = int(cnt2[c, ci, w])
                gseq = g_e[s0 : s0 + n_c]
                dseq = ds[s0 : s0 + n_c]
                for t in range(ntiles):
                    lo, hi = t * 128, min((t + 1) * 128, n_c)
                    if lo >= n_c:
                        break
                    for gv in np.unique(gseq[lo:hi]):
                        gv = int(gv)
                        sel = gseq[lo:hi] == gv
                        rels = dseq[lo:hi][sel] % GRP
                        r0, r1 = int(rels.min()), int(rels.max())
                        if gv in tile_groups[t]:
                            p0, p1 = tile_groups[t][gv]
                            tile_groups[t][gv] = (min(p0, r0), max(p1, r1))
                        else:
                            tile_groups[t][gv] = (r0, r1)
            t0 = 0
            while t0 < ntiles:
                tn = min(CAP_TILES, ntiles - t0)
                pairs = []
                for tl in range(tn):
                    for gv in sorted(tile_groups[t0 + tl]):
                        r0, r1 = tile_groups[t0 + tl][gv]
                        if not group_seen[gv]:
                            group_seen[gv] = True
                            r0 = 0
                        pairs.append((tl, gv, pcol, r0, r1 - r0 + 1))
                        pairs_per_group[gv] += 1
                        pcol += 1
                coh["calls"].append((w, icol, tn, t0, pairs))
                icol += tn * 8
                t0 += tn
        cohorts.append(coh)
    ICOLS = icol
    NPAIRS = pcol

    idx16_l, relp_l, degt_l = [], [], []
    ncol3 = (NG + 2) // 3
    for c in range(C):
        ds, sr, g_e = per_core[c]
        idx16 = np.zeros((128, ICOLS), dtype=np.int16)
        relp = np.full((128, NPAIRS), -1.0, dtype=np.float32)
        for ci, coh in enumerate(cohorts):
            for (w, icol0, tn, t0, pairs) in coh["calls"]:
                s0 = cell_pos[c][ci, w]
                n_c = int(cnt2[c, ci, w])
                nblk = tn * 128
                base = w * WSZ
                p_lo = t0 * 128
                p_hi = p_lo + nblk
                nreal = max(0, min(p_hi, n_c) - p_lo)
                locs = np.zeros(nblk, dtype=np.int64)
                if nreal > 0:
                    locs[:nreal] = sr[s0 + p_lo : s0 + p_lo + nreal] - base
                    locs[nreal:] = locs[nreal - 1] if nreal else 0
                elif n_c > 0:
                    locs[:] = sr[s0 + n_c - 1] - base
                wrapped = locs.astype(np.int16).reshape(nblk // 16, 16).T
                idx16[:, icol0 : icol0 + nblk // 16] = np.tile(wrapped, (8, 1))
                for (tl, gv, pc_, rlo, wp) in pairs:
                    lo = (t0 + tl) * 128
                    hi = min(lo + 128, n_c)
                    if hi <= lo:
                        continue
                    seg_g = g_e[s0 + lo : s0 + hi]
                    seg_d = ds[s0 + lo : s0 + hi]
                    sel = seg_g == gv
                    pp = np.nonzero(sel)[0]
                    relp[pp, pc_] = (seg_d[sel] % GRP - rlo).astype(np.float32)
        idx16_l.append(idx16)
        relp_l.append(np.ascontiguousarray(relp))

        # per-slot degree for the rank-1 BN fold: rows at partitions 0/32/64
        deg_p = np.bincount(ds, minlength=NG * GRP).astype(np.float32)
        deg_p[:SLOTS] += 1.0
        deg_p[SLOTS:] = 0.0
        dg = np.zeros((128, ncol3 * GRP), dtype=np.float32)
        for g in range(NG):
            dg[(g % 3) * 32, (g // 3) * GRP : (g // 3 + 1) * GRP] = deg_p[
                g * GRP : (g + 1) * GRP
            ]
        degt_l.append(dg.astype(np_bf16))

    x_tbl = np.zeros((TBL, 128), dtype=np_bf16)
    x_tbl[row_of, :IN_DIM] = x.astype(np_bf16)

    xT = []
    xs = x.astype(np.float32)
    for c in range(C):
        xt = np.zeros((128, SHP), dtype=np.float32)
        m = core_of == c
        xt[:IN_DIM, slot_loc[m]] = xs[m].T
        gsel = np.arange(N_GRAPHS)[(np.arange(N_GRAPHS) // GPC) == c]
        for g in gsel:
            sz = sizes[g]
            if sz < GS:
                base = (g - c * GPC) * GS
                xt[:IN_DIM, base + sz : base + GS] = xs[starts[g]][:, None]
        xT.append(xt.astype(np_bf16))

    return HostData(
        GS, GPC, SLOTS, NB, SHP, WSZ, NPAIRS, cohorts, pairs_per_group, ICOLS,
        idx16_l, relp_l, degt_l, x_tbl, np.array([0, NB]), xT,
    )


def build_program(hd: HostData):
    """Returns (nc, input_names)."""
    import concourse.bass as bass
    import concourse.mybir as mybir
    import concourse.tile as tile
    from concourse import bacc
    from concourse.masks import make_identity
    from concourse.tile_rust import add_dep_helper

    dt = mybir.dt
    Alu = mybir.AluOpType
    Act = mybir.ActivationFunctionType

    C, D = N_CORES, DIM
    NB, SHP, TBL, NG = hd.nb, hd.shp, hd.tbl, hd.ng
    GS, GPC, SLOTS, WSZ = hd.gs, hd.gpc, hd.slots, hd.wsz
    NPAIRS = hd.npairs
    inv_n = 1.0 / (C * SLOTS)

    nc = bacc.Bacc(
        "TRN2", target_bir_lowering=False, debug=False, num_devices=C
    )

    def din(name, shape, dtp=dt.float32):
        return nc.dram_tensor(name, list(shape), dtp, kind="ExternalInput").ap()

    x_tbl_d = din("x_tbl", (TBL, D), dt.bfloat16)
    xT_d = din("xT", (128, SHP), dt.bfloat16)
    idx_d = din("idx", (128, hd.icols), dt.int16)
    rel_d = din("rel", (128, NPAIRS))
    ncol3 = (NG + 2) // 3
    degt_d = din("degt", (128, ncol3 * GRP), dt.bfloat16)
    iota_d = din("iota", (128, GRP))
    w1_d = [din(f"w1_{l}", (D, D)) for l in range(3)]
    w2_d = [din(f"w2_{l}", (D, D)) for l in range(3)]
    b1_d = [din(f"b1_{l}", (D, 1)) for l in range(3)]
    b2_d = [din(f"b2_{l}", (D, 1)) for l in range(3)]
    gb_d = din("gb", (D, 6))
    out_d = nc.dram_tensor(
        "pooled", [GPC, 3 * D], dt.float32, kind="ExternalOutput"
    ).ap()

    input_names = (
        ["x_tbl", "xT", "idx", "rel", "degt", "iota"]
        + [f"w1_{l}" for l in range(3)]
        + [f"w2_{l}" for l in range(3)]
        + [f"b1_{l}" for l in range(3)]
        + [f"b2_{l}" for l in range(3)]
        + ["gb"]
    )

    n_pool_chunks = (GPC + 127) // 128
    last_chunk_rows = GPC - (n_pool_chunks - 1) * 128

    with tile.TileContext(nc) as tc:
        with (
            tc.tile_pool(name="const", bufs=1) as cpool,
            tc.tile_pool(name="ebuf", bufs=10) as epool,
            tc.tile_pool(name="spool", bufs=8) as spool,
            tc.tile_pool(name="zin", bufs=2) as zinpool,
            tc.tile_pool(name="zmid", bufs=2) as zmidpool,
            tc.tile_pool(name="rm", bufs=3) as rmpool,
            tc.tile_pool(name="stat", bufs=1) as statpool,
            tc.tile_pool(name="agg_ps", bufs=COH, space="PSUM") as aggpool,
            tc.tile_pool(name="m1_ps", bufs=1, space="PSUM") as m1pool,
            tc.tile_pool(name="m2_ps", bufs=2, space="PSUM") as m2pool,
            tc.tile_pool(name="tr_ps", bufs=1, space="PSUM") as trpool,
            tc.tile_pool(name="dram", bufs=1, space="DRAM") as dpool,
        ):
            # ---- DRAM intermediates ----
            cb = [int(v) for v in hd.cb]
            NCH = N_CHUNKS
            shq = [(cb[k + 1] - cb[k]) * 128 for k in range(NCH)]
            chunk_base = [0]
            for k in range(NCH):
                chunk_base.append(chunk_base[-1] + C * shq[k])
            h_tbl = [
                dpool.tile([TBL, D], dt.bfloat16, name=f"h_{l}", addr_space="Shared")
                for l in range(2)
            ]
            z_ch = [
                dpool.tile([shq[k], D], dt.bfloat16, name=f"z_ch{k}")
                for k in range(NCH)
            ]
            st_in = [
                dpool.tile([D, 2], dt.float32, name=f"st_in{l}") for l in range(3)
            ]
            st_out = [
                dpool.tile([D, 2], dt.float32, name=f"st_out{l}")
                for l in range(3)
            ]

            # ---- constants to SBUF ----
            def load(shape, src_ap, dtp=dt.float32, name=None):
                t = cpool.tile(list(shape), dtp, name=name)
                nc.sync.dma_start(out=t[:], in_=src_ap)
                return t

            idx_sb = load((128, hd.icols), idx_d[:], dt.int16, name="idx_sb")
            rel_sb = load((128, NPAIRS), rel_d[:], name="rel_sb")
            degt_sb = load(
                (128, ncol3 * GRP), degt_d[:], dt.bfloat16, name="degt_sb"
            )
            iota_sb = load((128, GRP), iota_d[:], name="iota_sb")
            w1_sb = [load((D, D), w1_d[l][:], name=f"w1sb{l}") for l in range(3)]
            w2_sb = [load((D, D), w2_d[l][:], name=f"w2sb{l}") for l in range(3)]
            b1_sb = [load((D, 1), b1_d[l][:], name=f"b1sb{l}") for l in range(3)]
            b2_sb = [load((D, 1), b2_d[l][:], name=f"b2sb{l}") for l in range(3)]
            gb_sb = load((D, 6), gb_d[:], name="gb_sb")
            w1r0 = cpool.tile([D, D], dt.float32, name="w1r0")
            nc.any.tensor_copy(out=w1r0[:], in_=w1_sb[0][:])
            w2r = []
            for l in range(3):
                t = cpool.tile([D, D], dt.float32, name=f"w2r{l}")
                nc.any.tensor_copy(out=t[:], in_=w2_sb[l][:])
                w2r.append(t)
            ident = cpool.tile([128, 128], dt.bfloat16, name="ident")
            make_identity(nc, ident[:])
            ident32 = cpool.tile([128, 128], dt.float32, name="ident32")
            make_identity(nc, ident32[:])

            s_all = cpool.tile([D, 3], dt.float32, name="s_all")
            t_all = cpool.tile([D, 3], dt.float32, name="t_all")
            w1s_sb = [
                cpool.tile([D, D], dt.float32, name=f"w1s{l}") for l in (1, 2)
            ]
            u_sb = [cpool.tile([1, D], dt.float32, name=f"u{l}") for l in (1, 2)]
            ub_sb = [
                cpool.tile([D, D], dt.bfloat16, name=f"ub{l}") for l in (1, 2)
            ]
            ones_row = cpool.tile([1, D], dt.float32, name="ones_row")
            nc.gpsimd.memset(ones_row[:], 1.0)
            ssum = cpool.tile([128, NG], dt.float32, name="ssum")
            ssq = cpool.tile([128, NG], dt.float32, name="ssq")
            sq_scr = cpool.tile([128, GRP], dt.float32, name="sq_scr")
            stat_scr = cpool.tile([128, 8], dt.float32, name="stat_scr")
            pt_all = [
                cpool.tile([128, GPC], dt.float32, name=f"pt{l}")
                for l in range(3)
            ]
            zkeep = cpool.tile([128, SHP], dt.bfloat16, name="zkeep")
            nc.sync.dma_start(out=zkeep[:], in_=xT_d[:])

            def compute_fold(l):
                st = statpool.tile([D, 2], dt.float32, name="st_ld")
                nc.sync.dma_start(out=st[:], in_=st_out[l][:])
                mu = stat_scr[:, 0:1]
                msq = stat_scr[:, 1:2]
                var = stat_scr[:, 2:3]
                rstd = stat_scr[:, 3:4]
                smu = stat_scr[:, 4:5]
                nc.vector.tensor_scalar_mul(mu, st[:, 0:1], inv_n)
                nc.vector.tensor_scalar_mul(msq, st[:, 1:2], inv_n)
                nc.vector.tensor_tensor(out=var, in0=mu, in1=mu, op=Alu.mult)
                nc.vector.tensor_tensor(
                    out=var, in0=msq, in1=var, op=Alu.subtract
                )
                veps = stat_scr[:, 6:7]
                nc.vector.tensor_scalar_add(veps, var, EPS)
                std = stat_scr[:, 5:6]
                nc.scalar.activation(std, veps, Act.Sqrt)
                nc.vector.reciprocal(rstd, std)
                scol = s_all[:, l : l + 1]
                tcol = t_all[:, l : l + 1]
                nc.vector.tensor_tensor(
                    out=scol, in0=gb_sb[:, 2 * l : 2 * l + 1], in1=rstd,
                    op=Alu.mult,
                )
                nc.vector.tensor_tensor(out=smu, in0=scol, in1=mu, op=Alu.mult)
                nc.vector.tensor_tensor(
                    out=tcol, in0=gb_sb[:, 2 * l + 1 : 2 * l + 2], in1=smu,
                    op=Alu.subtract,
                )
                if l < 2:
                    ln = l + 1
                    nc.vector.tensor_scalar(
                        out=w1s_sb[ln - 1][:], in0=w1_sb[ln][:], scalar1=scol,
                        scalar2=None, op0=Alu.mult,
                    )
                    ups = trpool.tile([1, D], dt.float32, name="ups", tag="tr")
                    nc.tensor.matmul(
                        ups[:], lhsT=tcol, rhs=w1_sb[ln][:], start=True,
                        stop=True,
                    )
                    nc.any.tensor_copy(out=u_sb[ln - 1][:], in_=ups[:])
                    ubp = trpool.tile([D, D], dt.float32, name="ubp", tag="tr")
                    nc.tensor.matmul(
                        ubp[:], lhsT=ones_row[:], rhs=u_sb[ln - 1][:],
                        start=True, stop=True,
                    )
                    nc.any.tensor_copy(out=ub_sb[ln - 1][:], in_=ubp[:])

            def win_ap(tensor_ap, w):
                wl = min(WSZ, TBL - w * WSZ)
                return tensor_ap[w * WSZ : w * WSZ + wl, :]

            ag_insts = [[], []]
            for layer in range(3):
                if layer > 0:
                    compute_fold(layer - 1)
                lhs1 = w1r0 if layer == 0 else w1s_sb[layer - 1]
                pt = pt_all[layer]
                tbl_ap = x_tbl_d if layer == 0 else h_tbl[layer - 1][:]

                pr_done = np.zeros(NG, dtype=np.int64)
                first_gather = True
                for coh in hd.cohorts:
                    aggt = {}
                    for (w, icol0, tn, t0, pairs) in coh["calls"]:
                        n = tn * 128
                        et = epool.tile(
                            [128, CAP_TILES * 128], dt.bfloat16, name="ebuf"
                        )
                        gi = nc.gpsimd.dma_gather(
                            et[:, :n].rearrange("p (t f) -> p t f", f=128),
                            win_ap(tbl_ap, w),
                            idx_sb[:, icol0 : icol0 + n // 16],
                            n,
                            n,
                            128,
                        )
                        if first_gather:
                            first_gather = False
                            if layer > 0:
                                for agi in ag_insts[layer - 1]:
                                    add_dep_helper(
                                        getattr(gi, "ins", gi),
                                        getattr(agi, "ins", agi),
                                        reason="gather waits h AllGather",
                                    )
                        for (tl, g, pc_, rlo, wp) in pairs:
                            Wg = min(GRP, SHP - g * GRP)
                            first = pr_done[g] == 0
                            r0, W = (0, Wg) if first else (rlo, wp)
                            if g not in aggt:
                                aggt[g] = aggpool.tile(
                                    [128, GRP], dt.float32, name="agg"
                                )
                            s_t = spool.tile(
                                [128, GRP], dt.bfloat16, name="s_t"
                            )
                            nc.vector.tensor_scalar(
                                out=s_t[:, :W], in0=iota_sb[:, :W],
                                scalar1=rel_sb[:, pc_ : pc_ + 1],
                                scalar2=None, op0=Alu.is_equal,
                            )
                            nc.tensor.matmul(
                                aggt[g][:, r0 : r0 + W],
                                lhsT=et[:, tl * 128 : (tl + 1) * 128],
                                rhs=s_t[:, :W],
                                start=first,
                                stop=(
                                    pr_done[g] + 1 == hd.pairs_per_group[g]
                                ),
                            )
                            pr_done[g] += 1
                    # ---- MLP on the cohort's groups ----
                    for g in coh["groups"]:
                        c0 = g * GRP
                        W = min(GRP, SHP - c0)
                        zin = zinpool.tile([128, GRP], dt.float32, name="zin")
                        nc.vector.tensor_tensor(
                            out=zin[:, :W], in0=aggt[g][:, :W],
                            in1=zkeep[:, c0 : c0 + W], op=Alu.add,
                        )
                        m1 = m1pool.tile([128, GRP], dt.float32, name="m1")
                        nc.tensor.matmul(
                            m1[:, :W], lhsT=lhs1[:], rhs=zin[:, :W],
                            start=True, stop=(layer == 0),
                        )
                        if layer > 0:
                            dp = (g % 3) * 32
                            dc = (g // 3) * GRP
                            nc.tensor.matmul(
                                m1[:, :W],
                                lhsT=ub_sb[layer - 1][dp : dp + 1, :],
                                rhs=degt_sb[dp : dp + 1, dc : dc + W],
                                start=False, stop=True,
                            )
                        z1 = zmidpool.tile([128, GRP], dt.float32, name="z1")
                        nc.scalar.activation(
                            z1[:, :W], m1[:, :W], Act.Relu, bias=b1_sb[layer][:]
                        )
                        m2 = m2pool.tile([128, GRP], dt.float32, name="m2")
                        nc.tensor.matmul(
                            m2[:, :W], lhsT=w2r[layer][:], rhs=z1[:, :W],
                            start=True, stop=True,
                        )
                        z2 = zkeep[:, c0 : c0 + W]
                        wr = min(W, max(0, SLOTS - c0))
                        if wr > 0:
                            nc.scalar.activation(
                                z2[:, :wr], m2[:, :wr], Act.Relu,
                                bias=b2_sb[layer][:],
                                accum_out=ssum[:, g : g + 1],
                            )
                        if wr < W:
                            nc.scalar.activation(
                                z2[:, wr:W], m2[:, wr:W], Act.Relu,
                                bias=b2_sb[layer][:],
                            )
                        if wr > 0:
                            nc.scalar.activation(
                                sq_scr[:, :wr], z2[:, :wr], Act.Square,
                                accum_out=ssq[:, g : g + 1],
                            )
                        # ---- on-the-fly pooling (raw m2; relu+b2 at end) ----
                        pc1 = min(c0 + W, SLOTS)
                        if c0 < pc1:
                            gfirst = (c0 + GS - 1) // GS
                            a = gfirst * GS - c0
                            gend = pc1 // GS
                            nfull = gend - gfirst
                            if nfull > 0:
                                nc.vector.tensor_reduce(
                                    out=pt[:, gfirst:gend],
                                    in_=m2[:, a : a + nfull * GS].rearrange(
                                        "p (g s) -> p g s", s=GS
                                    ),
                                    axis=mybir.AxisListType.X, op=Alu.max,
                                )
                            if a > 0:
                                la = min(a, pc1 - c0)
                                tmpm = stat_scr[:, 7:8]
                                nc.vector.tensor_reduce(
                                    out=tmpm, in_=m2[:, 0:la],
                                    axis=mybir.AxisListType.X, op=Alu.max,
                                )
                                gl = gfirst - 1
                                nc.vector.tensor_tensor(
                                    out=pt[:, gl : gl + 1],
                                    in0=pt[:, gl : gl + 1], in1=tmpm,
                                    op=Alu.max,
                                )
                            r0 = a + max(0, gend - gfirst) * GS
                            if gend >= gfirst and c0 + r0 < pc1:
                                nc.vector.tensor_reduce(
                                    out=pt[:, gend : gend + 1],
                                    in_=m2[:, r0 : pc1 - c0],
                                    axis=mybir.AxisListType.X, op=Alu.max,
                                )
                        # ---- transpose to node-major for the h table ----
                        if layer < 2:
                            for i in range(W // 128):
                                trp = trpool.tile(
                                    [128, 128], dt.bfloat16, name="trp",
                                    tag="tr",
                                )
                                nc.tensor.transpose(
                                    trp[:], z2[:, i * 128 : (i + 1) * 128],
                                    ident[:],
                                )
                                rm = rmpool.tile(
                                    [128, 128], dt.bfloat16, name="rm"
                                )
                                nc.any.tensor_copy(out=rm[:], in_=trp[:])
                                b2i = c0 // 128 + i
                                kch = 0
                                while cb[kch + 1] <= b2i:
                                    kch += 1
                                lr0 = (b2i - cb[kch]) * 128
                                nc.sync.dma_start(
                                    out=z_ch[kch][lr0 : lr0 + 128, :],
                                    in_=rm[:],
                                )
                            # launch chunk AllGather as soon as blocks done
                            for kch in range(NCH):
                                if (cb[kch + 1] - 1) * 128 // GRP == g:
                                    agi = nc.gpsimd.collective_compute(
                                        "AllGather", mybir.AluOpType.bypass,
                                        replica_groups=[list(range(C))],
                                        ins=[z_ch[kch].opt()],
                                        outs=[
                                            h_tbl[layer][
                                                chunk_base[kch] : chunk_base[
                                                    kch
                                                ]
                                                + C * shq[kch],
                                                :,
                                            ].opt()
                                        ],
                                    )
                                    ag_insts[layer].append(agi)

                # ---- stats reduce + AllReduce ----
                sp = statpool.tile([D, 2], dt.float32, name="sp")
                nc.vector.tensor_reduce(
                    out=sp[:, 0:1], in_=ssum[:, :NG],
                    axis=mybir.AxisListType.X, op=Alu.add,
                )
                nc.vector.tensor_reduce(
                    out=sp[:, 1:2], in_=ssq[:, :NG],
                    axis=mybir.AxisListType.X, op=Alu.add,
                )
                nc.sync.dma_start(out=st_in[layer][:], in_=sp[:])
                nc.gpsimd.collective_compute(
                    "AllReduce", mybir.AluOpType.add,
                    replica_groups=[list(range(C))],
                    ins=[st_in[layer].opt()], outs=[st_out[layer].opt()],
                )

            # ---- output: affine + transpose + store ----
            compute_fold(2)
            out_big = cpool.tile(
                [128, n_pool_chunks * 3 * D], dt.float32, name="out_big"
            )
            with tc.tile_pool(name="poolt", bufs=2) as ptpool:
                for l in range(3):
                    pre = ptpool.tile([128, GPC], dt.float32, name="pre")
                    nc.scalar.activation(
                        pre[:], pt_all[l][:], Act.Relu, bias=b2_sb[l][:]
                    )
                    pta = ptpool.tile([128, GPC], dt.float32, name="pta")
                    nc.vector.tensor_scalar(
                        out=pta[:], in0=pre[:],
                        scalar1=s_all[:, l : l + 1],
                        scalar2=t_all[:, l : l + 1], op0=Alu.mult, op1=Alu.add,
                    )
                    for ch in range(n_pool_chunks):
                        rows = (
                            128 if ch < n_pool_chunks - 1 else last_chunk_rows
                        )
                        trp = trpool.tile(
                            [128, 128], dt.float32, name="trpo", tag="tr"
                        )
                        nc.tensor.transpose(
                            trp[:rows, :],
                            pta[:, ch * 128 : ch * 128 + rows], ident32[:],
                        )
                        nc.any.tensor_copy(
                            out=out_big[
                                :rows, ch * 3 * D + l * D : ch * 3 * D
                                + (l + 1) * D
                            ],
                            in_=trp[:rows, :],
                        )
            for ch in range(n_pool_chunks):
                rows = 128 if ch < n_pool_chunks - 1 else last_chunk_rows
                nc.sync.dma_start(
                    out=out_d[ch * 128 : ch * 128 + rows, :],
                    in_=out_big[:rows, ch * 3 * D : (ch + 1) * 3 * D],
                )

    nc.compile()
    return nc, input_names


def make_in_maps(hd: HostData, inputs: dict, input_names):
    iota = np.tile(np.arange(GRP, dtype=np.float32), (128, 1))
    gb = np.zeros((DIM, 6), dtype=np.float32)
    for l in range(3):
        gb[:, 2 * l] = inputs["gamma"][l]
        gb[:, 2 * l + 1] = inputs["beta"][l]
    shared = {
        "x_tbl": hd.x_tbl,
        "iota": np.ascontiguousarray(iota),
        "gb": gb,
    }
    for l in range(3):
        w = np.zeros((DIM, DIM), dtype=np.float32)
        wl = inputs[f"w1_{l}"]
        w[: wl.shape[0], :] = wl
        shared[f"w1_{l}"] = w
        shared[f"w2_{l}"] = np.ascontiguousarray(
            inputs[f"w2_{l}"].astype(np.float32)
        )
        shared[f"b1_{l}"] = inputs[f"b1_{l}"].astype(np.float32).reshape(-1, 1)
        shared[f"b2_{l}"] = inputs[f"b2_{l}"].astype(np.float32).reshape(-1, 1)
    in_maps = []
    for c in range(N_CORES):
        m = dict(shared)
        m["idx"] = hd.idx16[c]
        m["rel"] = hd.relp[c]
        m["degt"] = hd.degt[c]
        m["xT"] = hd.xT[c]
        assert set(m.keys()) == set(input_names)
        in_maps.append(m)
    return in_maps


def _run_sharded_timed(nc, in_maps, n_cores, iters=10, warmup=2):
    """Execute the compiled Bass module via PJRT with device-resident inputs,
    timing `iters` back-to-back dispatches (excludes input upload/compile)."""
    import time

    import jax
    from jax.sharding import Mesh, NamedSharding, PartitionSpec
    from jax.experimental.shard_map import shard_map

    import concourse.mybir as mybir
    from concourse import bass2jax

    bass2jax.install_neuronx_cc_hook()
    partition_name = (
        nc.partition_id_tensor.name if nc.partition_id_tensor else None
    )
    in_names, out_names, out_avals, zero_outs = [], [], [], []
    for alloc in nc.m.functions[0].allocations:
        if not isinstance(alloc, mybir.MemoryLocationSet):
            continue
        name = alloc.memorylocations[0].name
        if alloc.kind == "ExternalInput":
            if name != partition_name:
                in_names.append(name)
        elif alloc.kind == "ExternalOutput":
            out_names.append(name)
            shape = tuple(alloc.tensor_shape)
            dtp = mybir.dt.np(alloc.dtype)
            out_avals.append(jax.core.ShapedArray(shape, dtp))
            zero_outs.append(np.zeros(shape, dtp))
    n_params, n_outs = len(in_names), len(out_avals)
    in_names.extend(out_names)
    if partition_name is not None:
        in_names.append(partition_name)
    donate = tuple(range(n_params, n_params + n_outs))

    def _body(*args):
        operands = list(args)
        if partition_name is not None:
            operands.append(bass2jax.partition_id_tensor())
        outs = bass2jax._bass_exec_p.bind(
            *operands,
            out_avals=tuple(out_avals),
            in_names=tuple(in_names),
            out_names=tuple(out_names),
            lowering_input_output_aliases=(),
            sim_require_finite=True,
            sim_require_nnan=True,
            nc=nc,
        )
        return tuple(outs)

    devices = jax.devices()[:n_cores]
    mesh = Mesh(np.asarray(devices), ("core",))
    pspec = PartitionSpec("core")
    in_specs = (pspec,) * (n_params + n_outs)
    sharded = jax.jit(
        shard_map(
            _body, mesh=mesh, in_specs=in_specs,
            out_specs=(pspec,) * len(out_names), check_rep=False,
        ),
        donate_argnums=donate, keep_unused=True,
    )
    shd = NamedSharding(mesh, pspec)
    per_core = [
        [np.asarray(m[name]) for name in in_names[:n_params]] for m in in_maps
    ]
    dev_in = [
        jax.device_put(
            np.concatenate([per_core[c][i] for c in range(n_cores)], axis=0),
            shd,
        )
        for i in range(n_params)
    ]
    n_calls = warmup + (iters if iters else 0)
    zsets = [
        [
            jax.device_put(
                np.zeros((n_cores * z.shape[0], *z.shape[1:]), z.dtype), shd
            )
            for z in zero_outs
        ]
        for _ in range(max(n_calls, 1))
    ]
    jax.block_until_ready(zsets)
    jax.block_until_ready(dev_in)
    outs = None
    for i in range(warmup):
        outs = sharded(*dev_in, *zsets[i])
        jax.block_until_ready(outs)
    dt = None
    if iters:
        t0 = time.perf_counter()
        ress = [sharded(*dev_in, *zsets[warmup + i]) for i in range(iters)]
        jax.block_until_ready(ress)
        dt = (time.perf_counter() - t0) / iters
        outs = ress[-1]
    if outs is None:
        outs = sharded(*dev_in, *zsets[0])
    results = [
        {
            name: np.asarray(outs[i]).reshape(n_cores, *out_avals[i].shape)[c]
            for i, name in enumerate(out_names)
        }
        for c in range(n_cores)
    ]
    return results, dt


def run(inputs: dict, timed: bool = False):
    x = np.asarray(inputs["x"])
    ei = np.asarray(inputs["edge_index"])
    batch = np.asarray(inputs["batch"])
    hd = prep_host(x, ei, batch)
    nc, input_names = build_program(hd)
    in_maps = make_in_maps(hd, inputs, input_names)
    results, dt = _run_sharded_timed(
        nc, in_maps, N_CORES,
        iters=(200 if timed else 0), warmup=(4 if timed else 1),
    )
    outs = [results[c]["pooled"] for c in range(N_CORES)]
    full = np.concatenate(outs, axis=0).astype(np.float32)
    return full, dt


def kernel(**inputs) -> np.ndarray:
    out, _ = run(inputs, timed=False)
    return out


# revision 22
# speedup vs baseline: 1.0884x; 1.0884x over previous
"""GIN (3-layer) message-passing kernel for Trainium2, 8 NeuronCores.

v2 — batched-gather rewrite of the graph-partition data-parallel design.

  - Graphs assigned to cores by id (750 graphs x 50 nodes per core); nodes
    renumbered into a chunk-interleaved shared h table (4 chunks per layer
    for overlapped AllGathers).  The GIN self term comes from an
    SBUF-resident feat-major copy (zkeep), BN is folded into the next
    layer's first matmul (scale + rank-1 degree correction), stats come
    free from activation accum_out and a 1KB AllReduce.
  - Aggregation (the v2 part): edges are sharded by destination core and
    grouped by (512-slot destination group, 32768-row source window).
    Each (cohort of 3 groups, window) produces one int16 dma_gather call
    (<=1024 indices) instead of per-128-edge indirect DMAs: the SWDGE
    offset walker only supports one offset column per indirect call
    (~1.3us/call measured), while dma_gather moves ~5-6ns/row.  Gathered
    k-tiles (128 edges) are reduced into per-group PSUM banks by one-hot
    matmuls ([128e,128f]^T x [128e,512slots], is_equal-built one-hots).
  - MLP runs in transposed space per 512-col group (fp32r), pooling
    on-the-fly from raw m2 (max commutes with the final monotone affine),
    transposes feed the next layer's node-major h table.
Host assembles the 8 per-core [750, 384] outputs into the full [6000, 384].
"""

import sys

sys.path.insert(0, "/opt/trn_rl_repo")

import math
from dataclasses import dataclass, field

import numpy as np

try:
    from ml_dtypes import bfloat16 as np_bf16
except ImportError:  # pragma: no cover
    import jax.numpy as _jnp

    np_bf16 = _jnp.bfloat16

N_GRAPHS = 6000
N_CORES = 8
IN_DIM = 77
DIM = 128
EPS = 1e-5
GRP = 512  # slots per PSUM aggregation group
COH = 2  # groups per cohort; 4 agg banks -> 2 cohorts in flight
NW = 10  # equal-size int16 gather windows (<=32768 rows each)
CAP_TILES = 8  # max k-tiles (128 idxs each) per dma_gather call
N_CHUNKS = 1  # single chunk: Shared DRAM allows one writer inst


@dataclass
class HostData:
    gs: int
    gpc: int
    slots: int
    nb: int
    shp: int
    wsz: int
    npairs: int
    cohorts: list  # per cohort: {'groups': [..], 'calls': [(w, icol0, tn, t0, pairs)]}
    pairs_per_group: np.ndarray  # [NG]
    icols: int
    idx16: list  # per core [128, icols] int16
    relp: list  # per core [128, npairs] f32
    degt: list  # per core [128, GRP] bf16
    x_tbl: np.ndarray  # [TBL, 128] bf16
    cb: np.ndarray  # chunk boundaries in blocks
    xT: list  # per core [128, SHP] bf16

    @property
    def tbl(self):
        return N_CORES * self.shp

    @property
    def ng(self):
        return (self.shp + GRP - 1) // GRP


def prep_host(x: np.ndarray, edge_index: np.ndarray, batch: np.ndarray) -> HostData:
    C = N_CORES
    N = x.shape[0]
    batch = batch.astype(np.int64)
    sizes = np.bincount(batch, minlength=N_GRAPHS)
    assert sizes.min() >= 1
    starts = np.concatenate([[0], np.cumsum(sizes)[:-1]])
    GS = int(sizes.max())
    GPC = N_GRAPHS // C
    SLOTS = GPC * GS
    NB = (SLOTS + 127) // 128
    SHP = NB * 128
    TBL = C * SHP
    NG = (SHP + GRP - 1) // GRP
    WSZ = ((TBL + NW - 1) // NW + 15) // 16 * 16
    assert WSZ <= 32768 and (NW - 1) * WSZ < TBL

    g_of = batch
    pos = np.arange(N, dtype=np.int64) - starts[g_of]
    core_of = g_of // GPC
    slot_loc = (g_of - core_of * GPC) * GS + pos
    row_of = (core_of * SHP + slot_loc).astype(np.int64)

    src = edge_index[0].astype(np.int64)
    dst = edge_index[1].astype(np.int64)

    e_core = [core_of[dst]]
    e_dslot = [slot_loc[dst]]
    e_srow = [row_of[src]]

    # duplicate slots: graph g's pad slots [size_g, GS) copy n0 = starts[g]
    n0_edges = np.where(dst == starts[g_of[dst]])[0]
    n0_g = g_of[dst[n0_edges]]
    max_pad = GS - int(sizes.min())
    for j in range(max_pad):
        gsel_mask = sizes + j < GS
        em = gsel_mask[n0_g]
        gg = n0_g[em]
        pc = gg // GPC
        ps = (gg - pc * GPC) * GS + sizes[gg] + j
        e_core.append(pc)
        e_dslot.append(ps)
        e_srow.append(row_of[src[n0_edges[em]]])

    e_core = np.concatenate(e_core)
    e_dslot = np.concatenate(e_dslot)
    e_srow = np.concatenate(e_srow)

    NCOH = (NG + COH - 1) // COH
    per_core = []
    cnt2 = np.zeros((C, NCOH, NW), dtype=np.int64)
    for c in range(C):
        m = e_core == c
        ds, sr = e_dslot[m], e_srow[m]
        g_e = ds // GRP
        coh_e = g_e // COH
        w_e = sr // WSZ
        order = np.lexsort((ds, w_e, coh_e))
        ds, sr, g_e, coh_e, w_e = (
            ds[order], sr[order], g_e[order], coh_e[order], w_e[order]
        )
        np.add.at(cnt2[c], (coh_e, w_e), 1)
        per_core.append((ds, sr, g_e))

    npad_cw = ((cnt2.max(axis=0) + 127) // 128) * 128  # [NCOH, NW]

    cell_pos = []
    for c in range(C):
        cc = cnt2[c].reshape(-1)
        cs = np.concatenate([[0], np.cumsum(cc)[:-1]]).reshape(NCOH, NW)
        cell_pos.append(cs)

    cohorts = []
    icol = 0
    pcol = 0
    pairs_per_group = np.zeros(NG, dtype=np.int64)
    group_seen = np.zeros(NG, dtype=bool)
    for ci in range(NCOH):
        groups = list(range(ci * COH, min((ci + 1) * COH, NG)))
        coh = {"groups": groups, "calls": []}
        for w in range(NW):
            npad = int(npad_cw[ci, w])
            if npad == 0:
                continue
            ntiles = npad // 128
            tile_groups = [dict() for _ in range(ntiles)]
            for c in range(C):
                ds, sr, g_e = per_core[c]
                s0 = cell_pos[c][ci, w]
                n_c = int(cnt2[c, ci, w])
                gseq = g_e[s0 : s0 + n_c]
                dseq = ds[s0 : s0 + n_c]
                for t in range(ntiles):
                    lo, hi = t * 128, min((t + 1) * 128, n_c)
                    if lo >= n_c:
                        break
                    for gv in np.unique(gseq[lo:hi]):
                        gv = int(gv)
                        sel = gseq[lo:hi] == gv
                        rels = dseq[lo:hi][sel] % GRP
                        r0, r1 = int(rels.min()), int(rels.max())
                        if gv in tile_groups[t]:
                            p0, p1 = tile_groups[t][gv]
                            tile_groups[t][gv] = (min(p0, r0), max(p1, r1))
                        else:
                            tile_groups[t][gv] = (r0, r1)
            t0 = 0
            while t0 < ntiles:
                tn = min(CAP_TILES, ntiles - t0)
                pairs = []
                for tl in range(tn):
                    for gv in sorted(tile_groups[t0 + tl]):
                        r0, r1 = tile_groups[t0 + tl][gv]
                        if not group_seen[gv]:
                            group_seen[gv] = True
                            r0 = 0
                        pairs.append((tl, gv, pcol, r0, r1 - r0 + 1))
                        pairs_per_group[gv] += 1
                        pcol += 1
                coh["calls"].append((w, icol, tn, t0, pairs))
                icol += tn * 8
                t0 += tn
        cohorts.append(coh)
    ICOLS = icol
    NPAIRS = pcol

    idx16_l, relp_l, degt_l = [], [], []
    ncol3 = (NG + 2) // 3
    for c in range(C):
        ds, sr, g_e = per_core[c]
        idx16 = np.zeros((128, ICOLS), dtype=np.int16)
        relp = np.full((128, NPAIRS), -1.0, dtype=np.float32)
        for ci, coh in enumerate(cohorts):
            for (w, icol0, tn, t0, pairs) in coh["calls"]:
                s0 = cell_pos[c][ci, w]
                n_c = int(cnt2[c, ci, w])
                nblk = tn * 128
                base = w * WSZ
                p_lo = t0 * 128
                p_hi = p_lo + nblk
                nreal = max(0, min(p_hi, n_c) - p_lo)
                locs = np.zeros(nblk, dtype=np.int64)
                if nreal > 0:
                    locs[:nreal] = sr[s0 + p_lo : s0 + p_lo + nreal] - base
                    locs[nreal:] = locs[nreal - 1] if nreal else 0
                elif n_c > 0:
                    locs[:] = sr[s0 + n_c - 1] - base
                wrapped = locs.astype(np.int16).reshape(nblk // 16, 16).T
                idx16[:, icol0 : icol0 + nblk // 16] = np.tile(wrapped, (8, 1))
                for (tl, gv, pc_, rlo, wp) in pairs:
                    lo = (t0 + tl) * 128
                    hi = min(lo + 128, n_c)
                    if hi <= lo:
                        continue
                    seg_g = g_e[s0 + lo : s0 + hi]
                    seg_d = ds[s0 + lo : s0 + hi]
                    sel = seg_g == gv
                    pp = np.nonzero(sel)[0]
                    relp[pp, pc_] = (seg_d[sel] % GRP - rlo).astype(np.float32)
        idx16_l.append(idx16)
        relp_l.append(np.ascontiguousarray(relp))

        # per-slot degree for the rank-1 BN fold: rows at partitions 0/32/64
        deg_p = np.bincount(ds, minlength=NG * GRP).astype(np.float32)
        deg_p[:SLOTS] += 1.0
        deg_p[SLOTS:] = 0.0
        dg = np.zeros((128, ncol3 * GRP), dtype=np.float32)
        for g in range(NG):
            dg[(g % 3) * 32, (g // 3) * GRP : (g // 3 + 1) * GRP] = deg_p[
                g * GRP : (g + 1) * GRP
            ]
        degt_l.append(dg.astype(np_bf16))

    x_tbl = np.zeros((TBL, 128), dtype=np_bf16)
    x_tbl[row_of, :IN_DIM] = x.astype(np_bf16)

    xT = []
    xs = x.astype(np.float32)
    for c in range(C):
        xt = np.zeros((128, SHP), dtype=np.float32)
        m = core_of == c
        xt[:IN_DIM, slot_loc[m]] = xs[m].T
        gsel = np.arange(N_GRAPHS)[(np.arange(N_GRAPHS) // GPC) == c]
        for g in gsel:
            sz = sizes[g]
            if sz < GS:
                base = (g - c * GPC) * GS
                xt[:IN_DIM, base + sz : base + GS] = xs[starts[g]][:, None]
        xT.append(xt.astype(np_bf16))

    return HostData(
        GS, GPC, SLOTS, NB, SHP, WSZ, NPAIRS, cohorts, pairs_per_group, ICOLS,
        idx16_l, relp_l, degt_l, x_tbl, np.array([0, NB]), xT,
    )


def build_program(hd: HostData):
    """Returns (nc, input_names)."""
    import concourse.bass as bass
    import concourse.mybir as mybir
    import concourse.tile as tile
    from concourse import bacc
    from concourse.masks import make_identity
    from concourse.tile_rust import add_dep_helper

    dt = mybir.dt
    Alu = mybir.AluOpType
    Act = mybir.ActivationFunctionType

    C, D = N_CORES, DIM
    NB, SHP, TBL, NG = hd.nb, hd.shp, hd.tbl, hd.ng
    GS, GPC, SLOTS, WSZ = hd.gs, hd.gpc, hd.slots, hd.wsz
    NPAIRS = hd.npairs
    inv_n = 1.0 / (C * SLOTS)

    nc = bacc.Bacc(
        "TRN2", target_bir_lowering=False, debug=False, num_devices=C
    )

    def din(name, shape, dtp=dt.float32):
        return nc.dram_tensor(name, list(shape), dtp, kind="ExternalInput").ap()

    x_tbl_d = din("x_tbl", (TBL, D), dt.bfloat16)
    xT_d = din("xT", (128, SHP), dt.bfloat16)
    idx_d = din("idx", (128, hd.icols), dt.int16)
    rel_d = din("rel", (128, NPAIRS))
    ncol3 = (NG + 2) // 3
    degt_d = din("degt", (128, ncol3 * GRP), dt.bfloat16)
    iota_d = din("iota", (128, GRP))
    w1_d = [din(f"w1_{l}", (D, D)) for l in range(3)]
    w2_d = [din(f"w2_{l}", (D, D)) for l in range(3)]
    b1_d = [din(f"b1_{l}", (D, 1)) for l in range(3)]
    b2_d = [din(f"b2_{l}", (D, 1)) for l in range(3)]
    gb_d = din("gb", (D, 6))
    out_d = nc.dram_tensor(
        "pooled", [GPC, 3 * D], dt.float32, kind="ExternalOutput"
    ).ap()

    input_names = (
        ["x_tbl", "xT", "idx", "rel", "degt", "iota"]
        + [f"w1_{l}" for l in range(3)]
        + [f"w2_{l}" for l in range(3)]
        + [f"b1_{l}" for l in range(3)]
        + [f"b2_{l}" for l in range(3)]
        + ["gb"]
    )

    n_pool_chunks = (GPC + 127) // 128
    last_chunk_rows = GPC - (n_pool_chunks - 1) * 128

    with tile.TileContext(nc) as tc:
        with (
            tc.tile_pool(name="const", bufs=1) as cpool,
            tc.tile_pool(name="ebuf", bufs=10) as epool,
            tc.tile_pool(name="spool", bufs=8) as spool,
            tc.tile_pool(name="zin", bufs=2) as zinpool,
            tc.tile_pool(name="zmid", bufs=2) as zmidpool,
            tc.tile_pool(name="rm", bufs=3) as rmpool,
            tc.tile_pool(name="stat", bufs=1) as statpool,
            tc.tile_pool(name="agg_ps", bufs=4, space="PSUM") as aggpool,
            tc.tile_pool(name="m1_ps", bufs=1, space="PSUM") as m1pool,
            tc.tile_pool(name="m2_ps", bufs=2, space="PSUM") as m2pool,
            tc.tile_pool(name="tr_ps", bufs=1, space="PSUM") as trpool,
            tc.tile_pool(name="dram", bufs=1, space="DRAM") as dpool,
        ):
            # ---- DRAM intermediates ----
            cb = [int(v) for v in hd.cb]
            NCH = N_CHUNKS
            shq = [(cb[k + 1] - cb[k]) * 128 for k in range(NCH)]
            chunk_base = [0]
            for k in range(NCH):
                chunk_base.append(chunk_base[-1] + C * shq[k])
            h_tbl = [
                dpool.tile([TBL, D], dt.bfloat16, name=f"h_{l}", addr_space="Shared")
                for l in range(2)
            ]
            z_ch = [
                dpool.tile([shq[k], D], dt.bfloat16, name=f"z_ch{k}")
                for k in range(NCH)
            ]
            st_in = [
                dpool.tile([D, 2], dt.float32, name=f"st_in{l}") for l in range(3)
            ]
            st_out = [
                dpool.tile([D, 2], dt.float32, name=f"st_out{l}")
                for l in range(3)
            ]

            # ---- constants to SBUF ----
            def load(shape, src_ap, dtp=dt.float32, name=None):
                t = cpool.tile(list(shape), dtp, name=name)
                nc.sync.dma_start(out=t[:], in_=src_ap)
                return t

            idx_sb = load((128, hd.icols), idx_d[:], dt.int16, name="idx_sb")
            rel_sb = load((128, NPAIRS), rel_d[:], name="rel_sb")
            degt_sb = load(
                (128, ncol3 * GRP), degt_d[:], dt.bfloat16, name="degt_sb"
            )
            iota_sb = load((128, GRP), iota_d[:], name="iota_sb")
            w1_sb = [load((D, D), w1_d[l][:], name=f"w1sb{l}") for l in range(3)]
            w2_sb = [load((D, D), w2_d[l][:], name=f"w2sb{l}") for l in range(3)]
            b1_sb = [load((D, 1), b1_d[l][:], name=f"b1sb{l}") for l in range(3)]
            b2_sb = [load((D, 1), b2_d[l][:], name=f"b2sb{l}") for l in range(3)]
            gb_sb = load((D, 6), gb_d[:], name="gb_sb")
            w1r0 = cpool.tile([D, D], dt.float32, name="w1r0")
            nc.any.tensor_copy(out=w1r0[:], in_=w1_sb[0][:])
            w2r = []
            for l in range(3):
                t = cpool.tile([D, D], dt.float32, name=f"w2r{l}")
                nc.any.tensor_copy(out=t[:], in_=w2_sb[l][:])
                w2r.append(t)
            ident = cpool.tile([128, 128], dt.bfloat16, name="ident")
            make_identity(nc, ident[:])
            ident32 = cpool.tile([128, 128], dt.float32, name="ident32")
            make_identity(nc, ident32[:])

            s_all = cpool.tile([D, 3], dt.float32, name="s_all")
            t_all = cpool.tile([D, 3], dt.float32, name="t_all")
            w1s_sb = [
                cpool.tile([D, D], dt.float32, name=f"w1s{l}") for l in (1, 2)
            ]
            u_sb = [cpool.tile([1, D], dt.float32, name=f"u{l}") for l in (1, 2)]
            ub_sb = [
                cpool.tile([D, D], dt.bfloat16, name=f"ub{l}") for l in (1, 2)
            ]
            ones_row = cpool.tile([1, D], dt.float32, name="ones_row")
            nc.gpsimd.memset(ones_row[:], 1.0)
            ssum = cpool.tile([128, NG], dt.float32, name="ssum")
            ssq = cpool.tile([128, NG], dt.float32, name="ssq")
            sq_scr = cpool.tile([128, GRP], dt.float32, name="sq_scr")
            stat_scr = cpool.tile([128, 8], dt.float32, name="stat_scr")
            pt_all = [
                cpool.tile([128, GPC], dt.float32, name=f"pt{l}")
                for l in range(3)
            ]
            zkeep = cpool.tile([128, SHP], dt.bfloat16, name="zkeep")
            nc.sync.dma_start(out=zkeep[:], in_=xT_d[:])

            def compute_fold(l):
                st = statpool.tile([D, 2], dt.float32, name="st_ld")
                nc.sync.dma_start(out=st[:], in_=st_out[l][:])
                mu = stat_scr[:, 0:1]
                msq = stat_scr[:, 1:2]
                var = stat_scr[:, 2:3]
                rstd = stat_scr[:, 3:4]
                smu = stat_scr[:, 4:5]
                nc.vector.tensor_scalar_mul(mu, st[:, 0:1], inv_n)
                nc.vector.tensor_scalar_mul(msq, st[:, 1:2], inv_n)
                nc.vector.tensor_tensor(out=var, in0=mu, in1=mu, op=Alu.mult)
                nc.vector.tensor_tensor(
                    out=var, in0=msq, in1=var, op=Alu.subtract
                )
                veps = stat_scr[:, 6:7]
                nc.vector.tensor_scalar_add(veps, var, EPS)
                std = stat_scr[:, 5:6]
                nc.scalar.activation(std, veps, Act.Sqrt)
                nc.vector.reciprocal(rstd, std)
                scol = s_all[:, l : l + 1]
                tcol = t_all[:, l : l + 1]
                nc.vector.tensor_tensor(
                    out=scol, in0=gb_sb[:, 2 * l : 2 * l + 1], in1=rstd,
                    op=Alu.mult,
                )
                nc.vector.tensor_tensor(out=smu, in0=scol, in1=mu, op=Alu.mult)
                nc.vector.tensor_tensor(
                    out=tcol, in0=gb_sb[:, 2 * l + 1 : 2 * l + 2], in1=smu,
                    op=Alu.subtract,
                )
                if l < 2:
                    ln = l + 1
                    nc.vector.tensor_scalar(
                        out=w1s_sb[ln - 1][:], in0=w1_sb[ln][:], scalar1=scol,
                        scalar2=None, op0=Alu.mult,
                    )
                    ups = trpool.tile([1, D], dt.float32, name="ups", tag="tr")
                    nc.tensor.matmul(
                        ups[:], lhsT=tcol, rhs=w1_sb[ln][:], start=True,
                        stop=True,
                    )
                    nc.any.tensor_copy(out=u_sb[ln - 1][:], in_=ups[:])
                    ubp = trpool.tile([D, D], dt.float32, name="ubp", tag="tr")
                    nc.tensor.matmul(
                        ubp[:], lhsT=ones_row[:], rhs=u_sb[ln - 1][:],
                        start=True, stop=True,
                    )
                    nc.any.tensor_copy(out=ub_sb[ln - 1][:], in_=ubp[:])

            def win_ap(tensor_ap, w):
                wl = min(WSZ, TBL - w * WSZ)
                return tensor_ap[w * WSZ : w * WSZ + wl, :]

            ag_insts = [[], []]
            for layer in range(3):
                if layer > 0:
                    compute_fold(layer - 1)
                lhs1 = w1r0 if layer == 0 else w1s_sb[layer - 1]
                pt = pt_all[layer]
                tbl_ap = x_tbl_d if layer == 0 else h_tbl[layer - 1][:]

                pr_done = np.zeros(NG, dtype=np.int64)
                first_gather = True
                for coh in hd.cohorts:
                    aggt = {}
                    for (w, icol0, tn, t0, pairs) in coh["calls"]:
                        n = tn * 128
                        et = epool.tile(
                            [128, CAP_TILES * 128], dt.bfloat16, name="ebuf"
                        )
                        gi = nc.gpsimd.dma_gather(
                            et[:, :n].rearrange("p (t f) -> p t f", f=128),
                            win_ap(tbl_ap, w),
                            idx_sb[:, icol0 : icol0 + n // 16],
                            n,
                            n,
                            128,
                        )
                        if first_gather:
                            first_gather = False
                            if layer > 0:
                                for agi in ag_insts[layer - 1]:
                                    add_dep_helper(
                                        getattr(gi, "ins", gi),
                                        getattr(agi, "ins", agi),
                                        reason="gather waits h AllGather",
                                    )
                        for (tl, g, pc_, rlo, wp) in pairs:
                            Wg = min(GRP, SHP - g * GRP)
                            first = pr_done[g] == 0
                            r0, W = (0, Wg) if first else (rlo, wp)
                            if g not in aggt:
                                aggt[g] = aggpool.tile(
                                    [128, GRP], dt.float32, name="agg"
                                )
                            s_t = spool.tile(
                                [128, GRP], dt.bfloat16, name="s_t"
                            )
                            nc.vector.tensor_scalar(
                                out=s_t[:, :W], in0=iota_sb[:, :W],
                                scalar1=rel_sb[:, pc_ : pc_ + 1],
                                scalar2=None, op0=Alu.is_equal,
                            )
                            nc.tensor.matmul(
                                aggt[g][:, r0 : r0 + W],
                                lhsT=et[:, tl * 128 : (tl + 1) * 128],
                                rhs=s_t[:, :W],
                                start=first,
                                stop=(
                                    pr_done[g] + 1 == hd.pairs_per_group[g]
                                ),
                            )
                            pr_done[g] += 1
                    # ---- MLP on the cohort's groups ----
                    for g in coh["groups"]:
                        c0 = g * GRP
                        W = min(GRP, SHP - c0)
                        zin = zinpool.tile([128, GRP], dt.float32, name="zin")
                        nc.vector.tensor_tensor(
                            out=zin[:, :W], in0=aggt[g][:, :W],
                            in1=zkeep[:, c0 : c0 + W], op=Alu.add,
                        )
                        m1 = m1pool.tile([128, GRP], dt.float32, name="m1")
                        nc.tensor.matmul(
                            m1[:, :W], lhsT=lhs1[:], rhs=zin[:, :W],
                            start=True, stop=(layer == 0),
                        )
                        if layer > 0:
                            dp = (g % 3) * 32
                            dc = (g // 3) * GRP
                            nc.tensor.matmul(
                                m1[:, :W],
                                lhsT=ub_sb[layer - 1][dp : dp + 1, :],
                                rhs=degt_sb[dp : dp + 1, dc : dc + W],
                                start=False, stop=True,
                            )
                        z1 = zmidpool.tile([128, GRP], dt.float32, name="z1")
                        nc.scalar.activation(
                            z1[:, :W], m1[:, :W], Act.Relu, bias=b1_sb[layer][:]
                        )
                        m2 = m2pool.tile([128, GRP], dt.float32, name="m2")
                        nc.tensor.matmul(
                            m2[:, :W], lhsT=w2r[layer][:], rhs=z1[:, :W],
                            start=True, stop=True,
                        )
                        z2 = zkeep[:, c0 : c0 + W]
                        wr = min(W, max(0, SLOTS - c0))
                        if wr > 0:
                            nc.scalar.activation(
                                z2[:, :wr], m2[:, :wr], Act.Relu,
                                bias=b2_sb[layer][:],
                                accum_out=ssum[:, g : g + 1],
                            )
                        if wr < W:
                            nc.scalar.activation(
                                z2[:, wr:W], m2[:, wr:W], Act.Relu,
                                bias=b2_sb[layer][:],
                            )
                        if wr > 0:
                            nc.scalar.activation(
                                sq_scr[:, :wr], z2[:, :wr], Act.Square,
                                accum_out=ssq[:, g : g + 1],
                            )
                        # ---- on-the-fly pooling (raw m2; relu+b2 at end) ----
                        pc1 = min(c0 + W, SLOTS)
                        if c0 < pc1:
                            gfirst = (c0 + GS - 1) // GS
                            a = gfirst * GS - c0
                            gend = pc1 // GS
                            nfull = gend - gfirst
                            if nfull > 0:
                                nc.vector.tensor_reduce(
                                    out=pt[:, gfirst:gend],
                                    in_=m2[:, a : a + nfull * GS].rearrange(
                                        "p (g s) -> p g s", s=GS
                                    ),
                                    axis=mybir.AxisListType.X, op=Alu.max,
                                )
                            if a > 0:
                                la = min(a, pc1 - c0)
                                tmpm = stat_scr[:, 7:8]
                                nc.vector.tensor_reduce(
                                    out=tmpm, in_=m2[:, 0:la],
                                    axis=mybir.AxisListType.X, op=Alu.max,
                                )
                                gl = gfirst - 1
                                nc.vector.tensor_tensor(
                                    out=pt[:, gl : gl + 1],
                                    in0=pt[:, gl : gl + 1], in1=tmpm,
                                    op=Alu.max,
                                )
                            r0 = a + max(0, gend - gfirst) * GS
                            if gend >= gfirst and c0 + r0 < pc1:
                                nc.vector.tensor_reduce(
                                    out=pt[:, gend : gend + 1],
                                    in_=m2[:, r0 : pc1 - c0],
                                    axis=mybir.AxisListType.X, op=Alu.max,
                                )
                        # ---- transpose to node-major for the h table ----
                        if layer < 2:
                            for i in range(W // 128):
                                trp = trpool.tile(
                                    [128, 128], dt.bfloat16, name="trp",
                                    tag="tr",
                                )
                                nc.tensor.transpose(
                                    trp[:], z2[:, i * 128 : (i + 1) * 128],
                                    ident[:],
                                )
                                rm = rmpool.tile(
                                    [128, 128], dt.bfloat16, name="rm"
                                )
                                nc.any.tensor_copy(out=rm[:], in_=trp[:])
                                b2i = c0 // 128 + i
                                kch = 0
                                while cb[kch + 1] <= b2i:
                                    kch += 1
                                lr0 = (b2i - cb[kch]) * 128
                                nc.sync.dma_start(
                                    out=z_ch[kch][lr0 : lr0 + 128, :],
                                    in_=rm[:],
                                )
                            # launch chunk AllGather as soon as blocks done
                            for kch in range(NCH):
                                if (cb[kch + 1] - 1) * 128 // GRP == g:
                                    agi = nc.gpsimd.collective_compute(
                                        "AllGather", mybir.AluOpType.bypass,
                                        replica_groups=[list(range(C))],
                                        ins=[z_ch[kch].opt()],
                                        outs=[
                                            h_tbl[layer][
                                                chunk_base[kch] : chunk_base[
                                                    kch
                                                ]
                                                + C * shq[kch],
                                                :,
                                            ].opt()
                                        ],
                                    )
                                    ag_insts[layer].append(agi)

                # ---- stats reduce + AllReduce ----
                sp = statpool.tile([D, 2], dt.float32, name="sp")
                nc.vector.tensor_reduce(
                    out=sp[:, 0:1], in_=ssum[:, :NG],
                    axis=mybir.AxisListType.X, op=Alu.add,
                )
                nc.vector.tensor_reduce(
                    out=sp[:, 1:2], in_=ssq[:, :NG],
                    axis=mybir.AxisListType.X, op=Alu.add,
                )
                nc.sync.dma_start(out=st_in[layer][:], in_=sp[:])
                nc.gpsimd.collective_compute(
                    "AllReduce", mybir.AluOpType.add,
                    replica_groups=[list(range(C))],
                    ins=[st_in[layer].opt()], outs=[st_out[layer].opt()],
                )

            # ---- output: affine + transpose + store ----
            compute_fold(2)
            out_big = cpool.tile(
                [128, n_pool_chunks * 3 * D], dt.float32, name="out_big"
            )
            with tc.tile_pool(name="poolt", bufs=2) as ptpool:
                for l in range(3):
                    pre = ptpool.tile([128, GPC], dt.float32, name="pre")
                    nc.scalar.activation(
                        pre[:], pt_all[l][:], Act.Relu, bias=b2_sb[l][:]
                    )
                    pta = ptpool.tile([128, GPC], dt.float32, name="pta")
                    nc.vector.tensor_scalar(
                        out=pta[:], in0=pre[:],
                        scalar1=s_all[:, l : l + 1],
                        scalar2=t_all[:, l : l + 1], op0=Alu.mult, op1=Alu.add,
                    )
                    for ch in range(n_pool_chunks):
                        rows = (
                            128 if ch < n_pool_chunks - 1 else last_chunk_rows
                        )
                        trp = trpool.tile(
                            [128, 128], dt.float32, name="trpo", tag="tr"
                        )
                        nc.tensor.transpose(
                            trp[:rows, :],
                            pta[:, ch * 128 : ch * 128 + rows], ident32[:],
                        )
                        nc.any.tensor_copy(
                            out=out_big[
                                :rows, ch * 3 * D + l * D : ch * 3 * D
                                + (l + 1) * D
                            ],
                            in_=trp[:rows, :],
                        )
            for ch in range(n_pool_chunks):
                rows = 128 if ch < n_pool_chunks - 1 else last_chunk_rows
                nc.sync.dma_start(
                    out=out_d[ch * 128 : ch * 128 + rows, :],
                    in_=out_big[:rows, ch * 3 * D : (ch + 1) * 3 * D],
                )

    nc.compile()
    return nc, input_names


def make_in_maps(hd: HostData, inputs: dict, input_names):
    iota = np.tile(np.arange(GRP, dtype=np.float32), (128, 1))
    gb = np.zeros((DIM, 6), dtype=np.float32)
    for l in range(3):
        gb[:, 2 * l] = inputs["gamma"][l]
        gb[:, 2 * l + 1] = inputs["beta"][l]
    shared = {
        "x_tbl": hd.x_tbl,
        "iota": np.ascontiguousarray(iota),
        "gb": gb,
    }
    for l in range(3):
        w = np.zeros((DIM, DIM), dtype=np.float32)
        wl = inputs[f"w1_{l}"]
        w[: wl.shape[0], :] = wl
        shared[f"w1_{l}"] = w
        shared[f"w2_{l}"] = np.ascontiguousarray(
            inputs[f"w2_{l}"].astype(np.float32)
        )
        shared[f"b1_{l}"] = inputs[f"b1_{l}"].astype(np.float32).reshape(-1, 1)
        shared[f"b2_{l}"] = inputs[f"b2_{l}"].astype(np.float32).reshape(-1, 1)
    in_maps = []
    for c in range(N_CORES):
        m = dict(shared)
        m["idx"] = hd.idx16[c]
        m["rel"] = hd.relp[c]
        m["degt"] = hd.degt[c]
        m["xT"] = hd.xT[c]
        assert set(m.keys()) == set(input_names)
        in_maps.append(m)
    return in_maps


def _run_sharded_timed(nc, in_maps, n_cores, iters=10, warmup=2):
    """Execute the compiled Bass module via PJRT with device-resident inputs,
    timing `iters` back-to-back dispatches (excludes input upload/compile)."""
    import time

    import jax
    from jax.sharding import Mesh, NamedSharding, PartitionSpec
    from jax.experimental.shard_map import shard_map

    import concourse.mybir as mybir
    from concourse import bass2jax

    bass2jax.install_neuronx_cc_hook()
    partition_name = (
        nc.partition_id_tensor.name if nc.partition_id_tensor else None
    )
    in_names, out_names, out_avals, zero_outs = [], [], [], []
    for alloc in nc.m.functions[0].allocations:
        if not isinstance(alloc, mybir.MemoryLocationSet):
            continue
        name = alloc.memorylocations[0].name
        if alloc.kind == "ExternalInput":
            if name != partition_name:
                in_names.append(name)
        elif alloc.kind == "ExternalOutput":
            out_names.append(name)
            shape = tuple(alloc.tensor_shape)
            dtp = mybir.dt.np(alloc.dtype)
            out_avals.append(jax.core.ShapedArray(shape, dtp))
            zero_outs.append(np.zeros(shape, dtp))
    n_params, n_outs = len(in_names), len(out_avals)
    in_names.extend(out_names)
    if partition_name is not None:
        in_names.append(partition_name)
    donate = tuple(range(n_params, n_params + n_outs))

    def _body(*args):
        operands = list(args)
        if partition_name is not None:
            operands.append(bass2jax.partition_id_tensor())
        outs = bass2jax._bass_exec_p.bind(
            *operands,
            out_avals=tuple(out_avals),
            in_names=tuple(in_names),
            out_names=tuple(out_names),
            lowering_input_output_aliases=(),
            sim_require_finite=True,
            sim_require_nnan=True,
            nc=nc,
        )
        return tuple(outs)

    devices = jax.devices()[:n_cores]
    mesh = Mesh(np.asarray(devices), ("core",))
    pspec = PartitionSpec("core")
    in_specs = (pspec,) * (n_params + n_outs)
    sharded = jax.jit(
        shard_map(
            _body, mesh=mesh, in_specs=in_specs,
            out_specs=(pspec,) * len(out_names), check_rep=False,
        ),
        donate_argnums=donate, keep_unused=True,
    )
    shd = NamedSharding(mesh, pspec)
    per_core = [
        [np.asarray(m[name]) for name in in_names[:n_params]] for m in in_maps
    ]
    dev_in = [
        jax.device_put(
            np.concatenate([per_core[c][i] for c in range(n_cores)], axis=0),
            shd,
        )
        for i in range(n_params)
    ]
    n_calls = warmup + (iters if iters else 0)
    zsets = [
        [
            jax.device_put(
                np.zeros((n_cores * z.shape[0], *z.shape[1:]), z.dtype), shd
            )
            for z in zero_outs
        ]
        for _ in range(max(n_calls, 1))
    ]
    jax.block_until_ready(zsets)
    jax.block_until_ready(dev_in)
    outs = None
    for i in range(warmup):
        outs = sharded(*dev_in, *zsets[i])
        jax.block_until_ready(outs)
    dt = None
    if iters:
        t0 = time.perf_counter()
        ress = [sharded(*dev_in, *zsets[warmup + i]) for i in range(iters)]
        jax.block_until_ready(ress)
        dt = (time.perf_counter() - t0) / iters
        outs = ress[-1]
    if outs is None:
        outs = sharded(*dev_in, *zsets[0])
    results = [
        {
            name: np.asarray(outs[i]).reshape(n_cores, *out_avals[i].shape)[c]
            for i, name in enumerate(out_names)
        }
        for c in range(n_cores)
    ]
    return results, dt


def run(inputs: dict, timed: bool = False):
    x = np.asarray(inputs["x"])
    ei = np.asarray(inputs["edge_index"])
    batch = np.asarray(inputs["batch"])
    hd = prep_host(x, ei, batch)
    nc, input_names = build_program(hd)
    in_maps = make_in_maps(hd, inputs, input_names)
    results, dt = _run_sharded_timed(
        nc, in_maps, N_CORES,
        iters=(200 if timed else 0), warmup=(4 if timed else 1),
    )
    outs = [results[c]["pooled"] for c in range(N_CORES)]
    full = np.concatenate(outs, axis=0).astype(np.float32)
    return full, dt


def kernel(**inputs) -> np.ndarray:
    out, _ = run(inputs, timed=False)
    return out
